# revision 12
# baseline (speedup 1.0000x reference)
"""Trainium2 Bass kernel for Memorynet (KNN-interp + 1x1-conv MLP).

V2.1. Pure data parallel over batch (32 batches -> 8 cores x 4).
Per 4-tile group (512 tokens), software-pipelined across groups:
  S: two row-packed K=32 bf16 matmul pairs (tile_position rows 0-31/32-63)
     -> S [128 tok, 512 n2] fp32 PSUM; DVE max8 + find_index8 per tile
     (DVE does nothing else).
  W: weight math on GPSIMD, reciprocal-free:
     w_k = prod_{j!=k} d_j / sum_k prod_{j!=k} d_j  (one TT-divide at the end)
  A: GPSIMD local_scatter builds A rows (2 tiles/call); sync-queue
     dma_start_transpose -> A.T [128 n2, 16x128 tok] bf16.
  MLP: recv+MLP feature-major bf16 matmuls with BN+ReLU folded into ScalarE
     activation; PE program order pipelined: S(g) | L1(g-2) | L2(g-3).
Host handles transposes / BN folding / g1 = f2 @ W1r.T / sharding.
"""

import sys

sys.path.insert(0, "/opt/trn_rl_repo")

import numpy as np
import ml_dtypes

import concourse.bass as bass
import concourse.bacc as bacc_mod
import concourse.mybir as mybir
from concourse.tile import TileContext
from concourse.bass_utils import run_bass_kernel_spmd

EPS_DIST = 1e-8
EPS_BN = 1e-5
NCORES = 8
BPC = 4
N1, N2, C1, C2 = 2048, 512, 128, 256
CIN, H1, H2 = C1 + C2, 256, 128
NT = N1 // 128
NG = NT // 4          # 4 groups per batch
NGG = BPC * NG        # 16 groups per core

f32 = mybir.dt.float32
bf16 = mybir.dt.bfloat16
u32 = mybir.dt.uint32
i16 = mybir.dt.int16


def build_bass():
    nc = bacc_mod.Bacc()
    p1p_d = nc.declare_dram_parameter("p1p", [BPC, 128, N1], bf16, isOutput=False)
    rhs_d = nc.declare_dram_parameter("rhsp", [BPC, 128, N2], bf16, isOutput=False)
    p1sq_d = nc.declare_dram_parameter("p1sq", [BPC, 128, NT], f32, isOutput=False)
    f1T_d = nc.declare_dram_parameter("f1T", [BPC, C1, N1], bf16, isOutput=False)
    g1_d = nc.declare_dram_parameter("g1r", [BPC, 128, 4 * H1], bf16, isOutput=False)
    W1f_d = nc.declare_dram_parameter("W1fT", [C1, H1], bf16, isOutput=False)
    W2T_d = nc.declare_dram_parameter("W2T", [H1, H2], bf16, isOutput=False)
    sb1_d = nc.declare_dram_parameter("sb1", [H1, 2], f32, isOutput=False)
    sb2_d = nc.declare_dram_parameter("sb2", [H2, 2], f32, isOutput=False)
    outT = nc.declare_dram_parameter("outT", [BPC, H2, N1], bf16, isOutput=True)

    AT = mybir.ActivationFunctionType
    OP = mybir.AluOpType

    with TileContext(nc) as tc:
        with (
            tc.tile_pool(name="const", bufs=1) as cpool,
            tc.tile_pool(name="batch", bufs=2) as bpool,
            tc.tile_pool(name="wm", bufs=4) as wpool,
            tc.tile_pool(name="ag", bufs=4) as apool,
            tc.tile_pool(name="att", bufs=4) as tpool,
            tc.tile_pool(name="xg", bufs=8) as xpool,
            tc.tile_pool(name="ps_s", bufs=4, space="PSUM") as ps_s,
            tc.tile_pool(name="ps_h1", bufs=2, space="PSUM") as ps_h1,
            tc.tile_pool(name="ps_l2", bufs=1, space="PSUM") as ps_l2,
        ):
            # ---- constants ----
            W1f = cpool.tile([C1, H1], bf16)
            nc.sync.dma_start(out=W1f[:], in_=W1f_d[:, :])
            W2T = [cpool.tile([128, H2], bf16, tag=f"w2_{k}", name=f"w2_{k}")
                   for k in range(2)]
            for k in range(2):
                nc.sync.dma_start(out=W2T[k][:], in_=W2T_d[128 * k:128 * (k + 1), :])
            sb1 = [cpool.tile([128, 2], f32, tag=f"sb1_{k}", name=f"sb1_{k}")
                   for k in range(2)]
            for k in range(2):
                nc.sync.dma_start(out=sb1[k][:], in_=sb1_d[128 * k:128 * (k + 1), :])
            sb2 = cpool.tile([128, 2], f32)
            nc.sync.dma_start(out=sb2[:], in_=sb2_d[:, :])

            bst = {}

            def load_batch(b):
                p1p = bpool.tile([128, N1], bf16, tag="p1p")
                nc.sync.dma_start(out=p1p[:], in_=p1p_d[b, :, :])
                rhsp = bpool.tile([128, N2], bf16, tag="rhsp")
                nc.sync.dma_start(out=rhsp[:], in_=rhs_d[b, :, :])
                p1sqb = bpool.tile([128, NT], f32, tag="p1sqb")
                nc.sync.dma_start(out=p1sqb[:], in_=p1sq_d[b, :, :])
                f1b = bpool.tile([C1, N1], bf16, tag="f1b")
                nc.scalar.dma_start(out=f1b[:], in_=f1T_d[b, :, :])
                g1sb = bpool.tile([128, 4, H1], bf16, tag="g1sb")
                nc.scalar.dma_start(
                    out=g1sb[:], in_=g1_d[b, :, :].rearrange("p (c d) -> p c d", c=4)
                )
                bst[b] = (p1p, rhsp, p1sqb, f1b, g1sb)

            gstate = {}

            def emit_S(gg):
                """S matmuls + top-8 + GPSIMD weight math + scatter + transpose
                for global group gg."""
                b, g = divmod(gg, NG)
                p1p, rhsp, p1sqb, f1b, g1sb = bst[b]
                maxg = wpool.tile([128, 4, 8], f32, tag="maxg")
                idxg = wpool.tile([128, 4, 8], mybir.dt.uint16, tag="idxg")
                Sp = []
                for tl in range(4):
                    t = 4 * g + tl
                    S = ps_s.tile([128, N2], f32, tag="Sp")
                    nc.tensor.matmul(
                        out=S[:],
                        lhsT=p1p[32 * tl:32 * (tl + 1), 128 * t:128 * (t + 1)],
                        rhs=rhsp[32 * tl:32 * (tl + 1), :],
                        start=True,
                        stop=True,
                        tile_position=(32 * tl, 0),
                    )
                    Sp.append(S)
                for tl in range(4):
                    nc.vector.max(out=maxg[:, tl, :], in_=Sp[tl][:])
                for tl in range(4):
                    nc.vector.max_index(
                        out=idxg[:, tl, :], in_max=maxg[:, tl, :],
                        in_values=Sp[tl][:]
                    )

                # ---- weight math fully on DVE (all deps DVE-local) ----
                dist = wpool.tile([128, 4, 4], f32, tag="dist")
                nc.vector.tensor_tensor(
                    out=dist[:],
                    in0=p1sqb[:, 4 * g:4 * (g + 1), None].to_broadcast([128, 4, 4]),
                    in1=maxg[:, :, 0:4],
                    op=OP.subtract,
                )
                nc.vector.tensor_scalar_max(dist[:], dist[:], 1e-8)
                recd = wpool.tile([128, 4, 4], f32, tag="recd")
                nc.vector.reciprocal(out=recd[:], in_=dist[:])
                Z = wpool.tile([128, 4], f32, tag="Z")
                nc.vector.reduce_sum(
                    out=Z[:], in_=recd[:, :, 0:3], axis=mybir.AxisListType.X
                )
                Zinv = wpool.tile([128, 4], f32, tag="Zinv")
                nc.vector.reciprocal(out=Zinv[:], in_=Z[:])
                wbf = wpool.tile([128, 4, 8], bf16, tag="wbf")
                nc.vector.tensor_tensor(
                    out=wbf[:, :, 0:3],
                    in0=recd[:, :, 0:3],
                    in1=Zinv[:, :, None].to_broadcast([128, 4, 3]),
                    op=OP.mult,
                )
                nc.vector.memset(wbf[:, :, 3:8], 0.0)
                nc.vector.tensor_scalar_add(
                    idxg[:, 1::2, :], idxg[:, 1::2, :], 512
                )
                gstate[("w", gg)] = (wbf, idxg)

            def emit_W(gg):
                """Scatter A rows on GPSIMD."""
                wbf, idxg = gstate.pop(("w", gg))
                Ag = apool.tile([128, 4, N2], bf16, tag="Ag")
                for h in range(2):
                    nc.gpsimd.local_scatter(
                        out_ap=Ag[:, 2 * h:2 * h + 2, :].rearrange("p t n -> p (t n)"),
                        data_ap=wbf[:, 2 * h:2 * h + 2, :].rearrange(
                            "p t k -> p (t k)"),
                        idxs_ap=idxg[:, 2 * h:2 * h + 2, :].rearrange(
                            "p t k -> p (t k)").bitcast(i16),
                        channels=128,
                        num_elems=2 * N2,
                        num_idxs=16,
                    )
                gstate[("a", gg)] = Ag

            def emit_T(gg):
                Ag = gstate.pop(("a", gg))
                ATt = tpool.tile([128, 16, 128], bf16, tag="ATt")
                nc.sync.dma_start_transpose(out=ATt[:], in_=Ag[:])
                gstate[gg] = ATt

            def emit_L1(gg):
                b, g = divmod(gg, NG)
                _, _, _, f1b, g1sb = bst[b]
                ATt = gstate[gg]
                ATv = ATt[:].rearrange("p (t c) r -> p c t r", c=4)
                h1 = [xpool.tile([128, 512], bf16, tag=f"h1_{m}", name=f"h1_{m}")
                      for m in range(2)]
                for m in range(2):
                    l1p = ps_h1.tile([128, 512], f32, tag="l1p")
                    for c in range(4):
                        nc.tensor.matmul(
                            out=l1p[:],
                            lhsT=g1sb[:, c, 128 * m:128 * (m + 1)],
                            rhs=ATv[:, c],
                            start=(c == 0),
                            stop=False,
                        )
                    nc.tensor.matmul(
                        out=l1p[:],
                        lhsT=W1f[:, 128 * m:128 * (m + 1)],
                        rhs=f1b[:, 512 * g:512 * (g + 1)],
                        start=False,
                        stop=True,
                    )
                    nc.scalar.activation(
                        out=h1[m][:],
                        in_=l1p[:],
                        func=AT.Relu,
                        scale=sb1[m][:, 0:1],
                        bias=sb1[m][:, 1:2],
                    )
                gstate[gg] = h1

            def emit_L2(gg):
                b, g = divmod(gg, NG)
                h1 = gstate.pop(gg)
                l2p = ps_l2.tile([128, 512], f32, tag="l2p")
                for kk in range(2):
                    nc.tensor.matmul(
                        out=l2p[:],
                        lhsT=W2T[kk][:],
                        rhs=h1[kk][:],
                        start=(kk == 0),
                        stop=(kk == 1),
                    )
                o = xpool.tile([128, 512], bf16, tag="osb")
                nc.scalar.activation(
                    out=o[:],
                    in_=l2p[:],
                    func=AT.Relu,
                    scale=sb2[:, 0:1],
                    bias=sb2[:, 1:2],
                )
                nc.scalar.dma_start(out=outT[b, :, 512 * g:512 * (g + 1)], in_=o[:])

            # software pipeline: W(g-1) | S(g) | T(g-2) | L1(g-4) | L2(g-5)
            load_batch(0)
            for gg in range(NGG + 5):
                if gg + 2 < NGG and (gg + 2) % NG == 0:
                    load_batch((gg + 2) // NG)
                if 0 <= gg - 1 < NGG:
                    emit_W(gg - 1)
                if gg < NGG:
                    emit_S(gg)
                if 0 <= gg - 2 < NGG:
                    emit_T(gg - 2)
                if 0 <= gg - 4 < NGG:
                    emit_L1(gg - 4)
                if 0 <= gg - 5 < NGG:
                    emit_L2(gg - 5)
    nc.compile()
    return nc


_CACHE = {}


def _get_nc():
    if "nc" not in _CACHE:
        _CACHE["nc"] = build_bass()
    return _CACHE["nc"]


def _prep_core(inputs, c):
    """Host-side prep of one core's input map (batches 4c..4c+4)."""
    sl = slice(BPC * c, BPC * (c + 1))
    p1 = inputs["points_1"][sl]
    p2 = inputs["points_2"][sl]
    f1 = inputs["features_1"][sl]
    f2 = inputs["features_2"][sl]

    def split3(x):
        a = x.astype(ml_dtypes.bfloat16)
        r = x - a.astype(np.float32)
        bb = r.astype(ml_dtypes.bfloat16)
        cc = (r - bb.astype(np.float32)).astype(ml_dtypes.bfloat16)
        return a, bb, cc

    p1T = np.transpose(p1, (0, 2, 1)).astype(np.float32)
    p2T2 = (2.0 * np.transpose(p2, (0, 2, 1))).astype(np.float32)
    p2sq = np.sum(p2.astype(np.float64) ** 2, -1)
    a1, b1_, c1_ = split3(p1T)
    x2, y2, z2 = split3(p2T2)
    s1_, s2_, s3_ = split3((-p2sq).astype(np.float32))
    onesr = np.ones((BPC, 1, N1), ml_dtypes.bfloat16)
    zpadl = np.zeros((BPC, 11, N1), ml_dtypes.bfloat16)
    p1e = np.concatenate(
        [a1, a1, b1_, a1, b1_, c1_, onesr, onesr, onesr, zpadl], axis=1
    )  # [4, 32, N1]
    p1pack = np.concatenate([p1e, p1e, p1e, p1e], axis=1)  # [4, 128, N1]
    zpadr = np.zeros((BPC, 11, N2), ml_dtypes.bfloat16)
    rhs4 = np.concatenate(
        [x2, y2, x2, z2, y2, x2,
         s1_[:, None, :], s2_[:, None, :], s3_[:, None, :], zpadr], axis=1
    )  # [4, 32, N2]
    rhspack = np.concatenate([rhs4, rhs4, rhs4, rhs4], axis=1)  # [4, 128, N2]
    p1sq = (np.sum(p1.astype(np.float64) ** 2, -1) + EPS_DIST).astype(np.float32)
    p1sqT = np.transpose(p1sq.reshape(BPC, NT, 128), (0, 2, 1))

    m = {
        "p1p": np.ascontiguousarray(p1pack.astype(ml_dtypes.bfloat16)),
        "rhsp": np.ascontiguousarray(rhspack.astype(ml_dtypes.bfloat16)),
        "p1sq": np.ascontiguousarray(p1sqT),
        "f1T": np.ascontiguousarray(
            np.transpose(f1, (0, 2, 1)).astype(ml_dtypes.bfloat16)
        ),
    }
    W1r = inputs["W1"][:, 0:C2]
    W1fT = inputs["W1"][:, C2:].T
    g1r = np.empty((BPC, 128, 4, H1), dtype=ml_dtypes.bfloat16)
    for b in range(BPC):
        g1b = f2[b].astype(np.float32) @ W1r.T.astype(np.float32)
        g1r[b] = g1b.reshape(4, 128, H1).transpose(1, 0, 2).astype(ml_dtypes.bfloat16)
    m["g1r"] = np.ascontiguousarray(g1r.reshape(BPC, 128, 4 * H1))
    m["W1fT"] = np.ascontiguousarray(W1fT.astype(ml_dtypes.bfloat16))
    s1 = inputs["g1"] / np.sqrt(inputs["v1"] + EPS_BN)
    b1f = (inputs["b1"] - inputs["m1"]) * s1 + inputs["be1"]
    s2 = inputs["g2"] / np.sqrt(inputs["v2"] + EPS_BN)
    b2f = (inputs["b2"] - inputs["m2"]) * s2 + inputs["be2"]
    m["W2T"] = np.ascontiguousarray(inputs["W2"].T.astype(ml_dtypes.bfloat16))
    m["sb1"] = np.ascontiguousarray(np.stack([s1, b1f], -1).astype(np.float32))
    m["sb2"] = np.ascontiguousarray(np.stack([s2, b2f], -1).astype(np.float32))
    return m


def run(inputs, trace=False):
    nc = _get_nc()
    in_maps = [_prep_core(inputs, c) for c in range(NCORES)]
    res = run_bass_kernel_spmd(
        nc, in_maps, core_ids=list(range(NCORES)), trace=trace
    )
    outs = [np.asarray(r["outT"]).astype(np.float32) for r in res.results]
    full = np.concatenate(outs, 0)
    out = np.ascontiguousarray(np.transpose(full, (0, 2, 1)))
    return out, res


def kernel(**inputs):
    out, _ = run(inputs, trace=False)
    return out


# revision 13
# speedup vs baseline: 1.0194x; 1.0194x over previous
"""Trainium2 Bass kernel for Memorynet (KNN-interp + 1x1-conv MLP).

V2.1. Pure data parallel over batch (32 batches -> 8 cores x 4).
Per 4-tile group (512 tokens), software-pipelined across groups:
  S: two row-packed K=32 bf16 matmul pairs (tile_position rows 0-31/32-63)
     -> S [128 tok, 512 n2] fp32 PSUM; DVE max8 + find_index8 per tile
     (DVE does nothing else).
  W: weight math on GPSIMD, reciprocal-free:
     w_k = prod_{j!=k} d_j / sum_k prod_{j!=k} d_j  (one TT-divide at the end)
  A: GPSIMD local_scatter builds A rows (2 tiles/call); sync-queue
     dma_start_transpose -> A.T [128 n2, 16x128 tok] bf16.
  MLP: recv+MLP feature-major bf16 matmuls with BN+ReLU folded into ScalarE
     activation; PE program order pipelined: S(g) | L1(g-2) | L2(g-3).
Host handles transposes / BN folding / g1 = f2 @ W1r.T / sharding.
"""

import sys

sys.path.insert(0, "/opt/trn_rl_repo")

import numpy as np
import ml_dtypes

import concourse.bass as bass
import concourse.bacc as bacc_mod
import concourse.mybir as mybir
from concourse.tile import TileContext
from concourse.bass_utils import run_bass_kernel_spmd

EPS_DIST = 1e-8
EPS_BN = 1e-5
NCORES = 8
BPC = 4
N1, N2, C1, C2 = 2048, 512, 128, 256
CIN, H1, H2 = C1 + C2, 256, 128
NT = N1 // 128
NG = NT // 4          # 4 groups per batch
NGG = BPC * NG        # 16 groups per core

f32 = mybir.dt.float32
bf16 = mybir.dt.bfloat16
u32 = mybir.dt.uint32
i16 = mybir.dt.int16


def build_bass():
    nc = bacc_mod.Bacc()
    p1p_d = nc.declare_dram_parameter("p1p", [BPC, 128, N1], bf16, isOutput=False)
    rhs_d = nc.declare_dram_parameter("rhsp", [BPC, 128, N2], bf16, isOutput=False)
    p1sq_d = nc.declare_dram_parameter("p1sq", [BPC, 128, NT], f32, isOutput=False)
    f1T_d = nc.declare_dram_parameter("f1T", [BPC, C1, N1], bf16, isOutput=False)
    g1_d = nc.declare_dram_parameter("g1r", [BPC, 128, 4 * H1], bf16, isOutput=False)
    W1f_d = nc.declare_dram_parameter("W1fT", [C1, H1], bf16, isOutput=False)
    W2T_d = nc.declare_dram_parameter("W2T", [H1, H2], bf16, isOutput=False)
    sb1_d = nc.declare_dram_parameter("sb1", [H1, 2], f32, isOutput=False)
    sb2_d = nc.declare_dram_parameter("sb2", [H2, 2], f32, isOutput=False)
    outT = nc.declare_dram_parameter("outT", [BPC, H2, N1], bf16, isOutput=True)

    AT = mybir.ActivationFunctionType
    OP = mybir.AluOpType

    with TileContext(nc) as tc:
        with (
            tc.tile_pool(name="const", bufs=1) as cpool,
            tc.tile_pool(name="batch", bufs=2) as bpool,
            tc.tile_pool(name="wm", bufs=4) as wpool,
            tc.tile_pool(name="ag", bufs=4) as apool,
            tc.tile_pool(name="att", bufs=4) as tpool,
            tc.tile_pool(name="xg", bufs=8) as xpool,
            tc.tile_pool(name="ps_s", bufs=4, space="PSUM") as ps_s,
            tc.tile_pool(name="ps_h1", bufs=2, space="PSUM") as ps_h1,
            tc.tile_pool(name="ps_l2", bufs=1, space="PSUM") as ps_l2,
        ):
            # ---- constants ----
            W1f = cpool.tile([C1, H1], bf16)
            nc.scalar.dma_start(out=W1f[:], in_=W1f_d[:, :])
            W2T = [cpool.tile([128, H2], bf16, tag=f"w2_{k}", name=f"w2_{k}")
                   for k in range(2)]
            for k in range(2):
                nc.scalar.dma_start(out=W2T[k][:], in_=W2T_d[128 * k:128 * (k + 1), :])
            sb1 = [cpool.tile([128, 2], f32, tag=f"sb1_{k}", name=f"sb1_{k}")
                   for k in range(2)]
            for k in range(2):
                nc.scalar.dma_start(out=sb1[k][:], in_=sb1_d[128 * k:128 * (k + 1), :])
            sb2 = cpool.tile([128, 2], f32)
            nc.scalar.dma_start(out=sb2[:], in_=sb2_d[:, :])

            bst = {}

            def load_batch(b):
                p1p = bpool.tile([128, N1], bf16, tag="p1p")
                nc.sync.dma_start(out=p1p[:], in_=p1p_d[b, :, :])
                rhsp = bpool.tile([128, N2], bf16, tag="rhsp")
                nc.sync.dma_start(out=rhsp[:], in_=rhs_d[b, :, :])
                p1sqb = bpool.tile([128, NT], f32, tag="p1sqb")
                nc.sync.dma_start(out=p1sqb[:], in_=p1sq_d[b, :, :])
                f1b = bpool.tile([C1, N1], bf16, tag="f1b")
                nc.scalar.dma_start(out=f1b[:], in_=f1T_d[b, :, :])
                g1sb = bpool.tile([128, 4, H1], bf16, tag="g1sb")
                nc.scalar.dma_start(
                    out=g1sb[:], in_=g1_d[b, :, :].rearrange("p (c d) -> p c d", c=4)
                )
                bst[b] = (p1p, rhsp, p1sqb, f1b, g1sb)

            gstate = {}

            def emit_S(gg):
                """S matmuls + top-8 + GPSIMD weight math + scatter + transpose
                for global group gg."""
                b, g = divmod(gg, NG)
                p1p, rhsp, p1sqb, f1b, g1sb = bst[b]
                maxg = wpool.tile([128, 4, 8], f32, tag="maxg")
                idxg = wpool.tile([128, 4, 8], mybir.dt.uint16, tag="idxg")
                Sp = []
                for tl in range(4):
                    t = 4 * g + tl
                    S = ps_s.tile([128, N2], f32, tag="Sp")
                    nc.tensor.matmul(
                        out=S[:],
                        lhsT=p1p[32 * tl:32 * (tl + 1), 128 * t:128 * (t + 1)],
                        rhs=rhsp[32 * tl:32 * (tl + 1), :],
                        start=True,
                        stop=True,
                        tile_position=(32 * tl, 0),
                    )
                    Sp.append(S)
                for tl in range(4):
                    nc.vector.max(out=maxg[:, tl, :], in_=Sp[tl][:])
                for tl in range(4):
                    nc.vector.max_index(
                        out=idxg[:, tl, :], in_max=maxg[:, tl, :],
                        in_values=Sp[tl][:]
                    )

                # ---- weight math fully on DVE (all deps DVE-local) ----
                dist = wpool.tile([128, 4, 4], f32, tag="dist")
                nc.vector.tensor_tensor(
                    out=dist[:],
                    in0=p1sqb[:, 4 * g:4 * (g + 1), None].to_broadcast([128, 4, 4]),
                    in1=maxg[:, :, 0:4],
                    op=OP.subtract,
                )
                nc.vector.tensor_scalar_max(dist[:], dist[:], 1e-8)
                recd = wpool.tile([128, 4, 4], f32, tag="recd")
                nc.vector.reciprocal(out=recd[:], in_=dist[:])
                Z = wpool.tile([128, 4], f32, tag="Z")
                nc.vector.reduce_sum(
                    out=Z[:], in_=recd[:, :, 0:3], axis=mybir.AxisListType.X
                )
                Zinv = wpool.tile([128, 4], f32, tag="Zinv")
                nc.vector.reciprocal(out=Zinv[:], in_=Z[:])
                wbf = wpool.tile([128, 4, 8], bf16, tag="wbf")
                nc.vector.tensor_tensor(
                    out=wbf[:, :, 0:3],
                    in0=recd[:, :, 0:3],
                    in1=Zinv[:, :, None].to_broadcast([128, 4, 3]),
                    op=OP.mult,
                )
                nc.vector.memset(wbf[:, :, 3:8], 0.0)
                nc.vector.tensor_scalar_add(
                    idxg[:, 1::2, :], idxg[:, 1::2, :], 512
                )
                gstate[("w", gg)] = (wbf, idxg)

            def emit_W(gg):
                """Scatter A rows on GPSIMD."""
                wbf, idxg = gstate.pop(("w", gg))
                Ag = apool.tile([128, 4, N2], bf16, tag="Ag")
                for h in range(2):
                    nc.gpsimd.local_scatter(
                        out_ap=Ag[:, 2 * h:2 * h + 2, :].rearrange("p t n -> p (t n)"),
                        data_ap=wbf[:, 2 * h:2 * h + 2, :].rearrange(
                            "p t k -> p (t k)"),
                        idxs_ap=idxg[:, 2 * h:2 * h + 2, :].rearrange(
                            "p t k -> p (t k)").bitcast(i16),
                        channels=128,
                        num_elems=2 * N2,
                        num_idxs=16,
                    )
                gstate[("a", gg)] = Ag

            def emit_T(gg):
                Ag = gstate.pop(("a", gg))
                ATt = tpool.tile([128, 16, 128], bf16, tag="ATt")
                nc.sync.dma_start_transpose(out=ATt[:], in_=Ag[:])
                gstate[gg] = ATt

            def emit_L1(gg):
                b, g = divmod(gg, NG)
                _, _, _, f1b, g1sb = bst[b]
                ATt = gstate[gg]
                ATv = ATt[:].rearrange("p (t c) r -> p c t r", c=4)
                h1 = [xpool.tile([128, 512], bf16, tag=f"h1_{m}", name=f"h1_{m}")
                      for m in range(2)]
                for m in range(2):
                    l1p = ps_h1.tile([128, 512], f32, tag="l1p")
                    for c in range(4):
                        nc.tensor.matmul(
                            out=l1p[:],
                            lhsT=g1sb[:, c, 128 * m:128 * (m + 1)],
                            rhs=ATv[:, c],
                            start=(c == 0),
                            stop=False,
                        )
                    nc.tensor.matmul(
                        out=l1p[:],
                        lhsT=W1f[:, 128 * m:128 * (m + 1)],
                        rhs=f1b[:, 512 * g:512 * (g + 1)],
                        start=False,
                        stop=True,
                    )
                    nc.scalar.activation(
                        out=h1[m][:],
                        in_=l1p[:],
                        func=AT.Relu,
                        scale=sb1[m][:, 0:1],
                        bias=sb1[m][:, 1:2],
                    )
                gstate[gg] = h1

            def emit_L2(gg):
                b, g = divmod(gg, NG)
                h1 = gstate.pop(gg)
                l2p = ps_l2.tile([128, 512], f32, tag="l2p")
                for kk in range(2):
                    nc.tensor.matmul(
                        out=l2p[:],
                        lhsT=W2T[kk][:],
                        rhs=h1[kk][:],
                        start=(kk == 0),
                        stop=(kk == 1),
                    )
                o = xpool.tile([128, 512], bf16, tag="osb")
                nc.scalar.activation(
                    out=o[:],
                    in_=l2p[:],
                    func=AT.Relu,
                    scale=sb2[:, 0:1],
                    bias=sb2[:, 1:2],
                )
                nc.scalar.dma_start(out=outT[b, :, 512 * g:512 * (g + 1)], in_=o[:])

            # software pipeline: W(g-1) | S(g) | T(g-1) | L1(g-2) | L2(g-3)
            load_batch(0)
            for gg in range(NGG + 3):
                if gg + 2 < NGG and (gg + 2) % NG == 0:
                    load_batch((gg + 2) // NG)
                if 0 <= gg - 1 < NGG:
                    emit_W(gg - 1)
                if gg < NGG:
                    emit_S(gg)
                if 0 <= gg - 1 < NGG:
                    emit_T(gg - 1)
                if 0 <= gg - 2 < NGG:
                    emit_L1(gg - 2)
                if 0 <= gg - 3 < NGG:
                    emit_L2(gg - 3)
    nc.compile()
    return nc


_CACHE = {}


def _get_nc():
    if "nc" not in _CACHE:
        _CACHE["nc"] = build_bass()
    return _CACHE["nc"]


def _prep_core(inputs, c):
    """Host-side prep of one core's input map (batches 4c..4c+4)."""
    sl = slice(BPC * c, BPC * (c + 1))
    p1 = inputs["points_1"][sl]
    p2 = inputs["points_2"][sl]
    f1 = inputs["features_1"][sl]
    f2 = inputs["features_2"][sl]

    def split3(x):
        a = x.astype(ml_dtypes.bfloat16)
        r = x - a.astype(np.float32)
        bb = r.astype(ml_dtypes.bfloat16)
        cc = (r - bb.astype(np.float32)).astype(ml_dtypes.bfloat16)
        return a, bb, cc

    p1T = np.transpose(p1, (0, 2, 1)).astype(np.float32)
    p2T2 = (2.0 * np.transpose(p2, (0, 2, 1))).astype(np.float32)
    p2sq = np.sum(p2.astype(np.float64) ** 2, -1)
    a1, b1_, c1_ = split3(p1T)
    x2, y2, z2 = split3(p2T2)
    s1_, s2_, s3_ = split3((-p2sq).astype(np.float32))
    onesr = np.ones((BPC, 1, N1), ml_dtypes.bfloat16)
    zpadl = np.zeros((BPC, 11, N1), ml_dtypes.bfloat16)
    p1e = np.concatenate(
        [a1, a1, b1_, a1, b1_, c1_, onesr, onesr, onesr, zpadl], axis=1
    )  # [4, 32, N1]
    p1pack = np.concatenate([p1e, p1e, p1e, p1e], axis=1)  # [4, 128, N1]
    zpadr = np.zeros((BPC, 11, N2), ml_dtypes.bfloat16)
    rhs4 = np.concatenate(
        [x2, y2, x2, z2, y2, x2,
         s1_[:, None, :], s2_[:, None, :], s3_[:, None, :], zpadr], axis=1
    )  # [4, 32, N2]
    rhspack = np.concatenate([rhs4, rhs4, rhs4, rhs4], axis=1)  # [4, 128, N2]
    p1sq = (np.sum(p1.astype(np.float64) ** 2, -1) + EPS_DIST).astype(np.float32)
    p1sqT = np.transpose(p1sq.reshape(BPC, NT, 128), (0, 2, 1))

    m = {
        "p1p": np.ascontiguousarray(p1pack.astype(ml_dtypes.bfloat16)),
        "rhsp": np.ascontiguousarray(rhspack.astype(ml_dtypes.bfloat16)),
        "p1sq": np.ascontiguousarray(p1sqT),
        "f1T": np.ascontiguousarray(
            np.transpose(f1, (0, 2, 1)).astype(ml_dtypes.bfloat16)
        ),
    }
    W1r = inputs["W1"][:, 0:C2]
    W1fT = inputs["W1"][:, C2:].T
    g1r = np.empty((BPC, 128, 4, H1), dtype=ml_dtypes.bfloat16)
    for b in range(BPC):
        g1b = f2[b].astype(np.float32) @ W1r.T.astype(np.float32)
        g1r[b] = g1b.reshape(4, 128, H1).transpose(1, 0, 2).astype(ml_dtypes.bfloat16)
    m["g1r"] = np.ascontiguousarray(g1r.reshape(BPC, 128, 4 * H1))
    m["W1fT"] = np.ascontiguousarray(W1fT.astype(ml_dtypes.bfloat16))
    s1 = inputs["g1"] / np.sqrt(inputs["v1"] + EPS_BN)
    b1f = (inputs["b1"] - inputs["m1"]) * s1 + inputs["be1"]
    s2 = inputs["g2"] / np.sqrt(inputs["v2"] + EPS_BN)
    b2f = (inputs["b2"] - inputs["m2"]) * s2 + inputs["be2"]
    m["W2T"] = np.ascontiguousarray(inputs["W2"].T.astype(ml_dtypes.bfloat16))
    m["sb1"] = np.ascontiguousarray(np.stack([s1, b1f], -1).astype(np.float32))
    m["sb2"] = np.ascontiguousarray(np.stack([s2, b2f], -1).astype(np.float32))
    return m


def run(inputs, trace=False):
    nc = _get_nc()
    in_maps = [_prep_core(inputs, c) for c in range(NCORES)]
    res = run_bass_kernel_spmd(
        nc, in_maps, core_ids=list(range(NCORES)), trace=trace
    )
    outs = [np.asarray(r["outT"]).astype(np.float32) for r in res.results]
    full = np.concatenate(outs, 0)
    out = np.ascontiguousarray(np.transpose(full, (0, 2, 1)))
    return out, res


def kernel(**inputs):
    out, _ = run(inputs, trace=False)
    return out
